# revision 36
# baseline (speedup 1.0000x reference)
"""Multi-head attention (B=2, S=2048, H=1024, NH=16) on 8 TRN2 NeuronCores.

Sharding: fully data/tensor parallel, no collectives. Core c = (b, hg) with
b = c // 4 (batch), hg = c % 4 (head group of 4 heads = 256 of the 1024
projection output dims). Each core:
  - gets its batch's query/key/value pre-transposed on host to [H, S] bf16
    (contraction dim on SBUF partitions, fast contiguous DMA, half traffic),
  - projects qT/kT [256, S] and v [S, 256] with its slice of Wq/Wk/Wv,
  - runs flash-style attention per head entirely on-chip:
      scoresT[k, q] = kT_h.T @ qT_h   (PE, bf16 in / f32 PSUM out)
      p = exp(scores/8 + mask[k])      (ACT; no max-subtraction needed:
                                        scores are O(1) by construction)
      ctxT_unnorm[65, q] = [v_h | 1].T @ p  (PE; row 64 = softmax denom)
      ctxT = ctxT_unnorm / bcast(denom)     (PE ones-bcast + DVE divide)
  - writes ctxT [256, S] f32; host transposes + scatters into [B, S, H].
Attention inner loops run kc-outer / qb-inner so consecutive matmuls share
the stationary operand (fewer LDWEIGHTS).
"""

import functools
import sys

if "/opt/trn_rl_repo" not in sys.path:
    sys.path.insert(0, "/opt/trn_rl_repo")

import numpy as np

B, S, H = 2, 2048, 1024
NH, HD = 16, 64
NCORES = 8
GROUPS = 4                # head groups (cores per batch)
DPG = H // GROUPS         # projection dims per core = 256
HPG = DPG // HD           # heads per core = 4
P = 128                   # SBUF partitions
NHC = H // P              # contraction chunks per projection = 8
QB = 512                  # q block (matmul moving free dim)
NQB = S // QB             # 4
NKC = S // P              # k chunks = 16
VA_W = HD + 1             # v_aug cols per head (64 v dims + ones col)
VA_PAD = 128              # slot width padded so LDWEIGHTS gets FWL


@functools.lru_cache(maxsize=1)
def _build():
    import concourse.bacc as bacc
    import concourse.mybir as mybir
    import concourse.tile as tile
    from concourse.tile_rust import add_dep_helper

    F32 = mybir.dt.float32
    F32R = mybir.dt.float32r
    BF16 = mybir.dt.bfloat16
    Exp = mybir.ActivationFunctionType.Exp
    MULT = mybir.AluOpType.mult
    ADD = mybir.AluOpType.add

    nc = bacc.Bacc()

    xq_d = nc.declare_dram_parameter("xq", [H, S], BF16, isOutput=False)
    xk_d = nc.declare_dram_parameter("xk", [H, S], BF16, isOutput=False)
    xv_d = nc.declare_dram_parameter("xv", [H, S], BF16, isOutput=False)
    wq_d = nc.declare_dram_parameter("wq", [H, DPG], BF16, isOutput=False)
    wk_d = nc.declare_dram_parameter("wk", [H, DPG], BF16, isOutput=False)
    wv_d = nc.declare_dram_parameter("wv", [H, DPG], BF16, isOutput=False)
    bqk_d = nc.declare_dram_parameter("bqk", [P, 6], F32, isOutput=False)
    mk_d = nc.declare_dram_parameter("mk", [P, NKC], F32, isOutput=False)
    out_d = nc.declare_dram_parameter("out", [S, DPG], F32, isOutput=True)
    id_d = nc.declare_dram_parameter("ident", [P, P], F32, isOutput=False)

    NINJ = 20  # scores rounds injected into the q/v projection phase

    with tile.TileContext(nc) as tc:
        with (
            tc.tile_pool(name="const", bufs=1) as cpool,
            tc.tile_pool(name="proj_out", bufs=1) as projpool,
            tc.tile_pool(name="xt", bufs=6) as xpool,
            tc.tile_pool(name="pexp", bufs=26) as ppool,
            tc.tile_pool(name="small", bufs=3) as spool,
        ):
            # ---- constants ----
            bqk_sb = cpool.tile([P, 6], F32)    # cols: q0 q1 k0 k1 v0 v1
            nc.sync.dma_start(bqk_sb[:], bqk_d[:])
            mk_sb = cpool.tile([P, NKC], F32)
            nc.sync.dma_start(mk_sb[:], mk_d[:])
            id_sb = cpool.tile([P, P], F32)
            nc.sync.dma_start(id_sb[:], id_d[:])
            id_bf = cpool.tile([P, P], BF16)
            nc.vector.tensor_copy(id_bf[:], id_sb[:])

            wq_sb = cpool.tile([P, NHC * DPG], BF16)
            wk_sb = cpool.tile([P, NHC * DPG], BF16)
            wv_sb = cpool.tile([P, NHC * DPG], BF16)

            # ---- persistent projection outputs ----
            qT0 = projpool.tile([P, S], BF16)
            qT1 = projpool.tile([P, S], BF16)
            kT0 = projpool.tile([P, S], BF16)
            kT1 = projpool.tile([P, S], BF16)
            vT0 = projpool.tile([P, S], BF16)
            vT1 = projpool.tile([P, S], BF16)
            va_sb = projpool.tile([P, NKC * HPG * VA_PAD], BF16)
            nc.vector.memset(va_sb[:], 0.0)
            for sc in range(NKC):
                for h in range(HPG):
                    oc = (sc * HPG + h) * VA_PAD + HD
                    nc.vector.memset(va_sb[:, oc : oc + 1], 1.0)

            rounds = [
                (h, pr, kc)
                for pr in range(NQB // 2)
                for h in range(HPG)
                for kc in range(NKC)
            ]
            NR = len(rounds)
            pq = [None] * NR

            def proj_pair(x_d, w_sb, bcol, dst0, dst1, pr, psA,
                          inject=None):
                cols0 = pr * 2 * QB
                pp = [
                    psA.tile([P, QB], F32, tag=f"pp{j}", name=f"pp{j}",
                             bufs=1)
                    for j in range(4)
                ]
                for hc in range(NHC):
                    if inject is not None:
                        inject(hc)
                    xt = xpool.tile([P, 2 * QB], BF16, tag="xt")
                    nc.sync.dma_start(
                        xt[:, :QB],
                        x_d[hc * P : (hc + 1) * P, cols0 : cols0 + QB],
                    )
                    nc.sync.dma_start(
                        xt[:, QB:],
                        x_d[hc * P : (hc + 1) * P,
                            cols0 + QB : cols0 + 2 * QB],
                    )
                    st = dict(start=(hc == 0), stop=(hc == NHC - 1))
                    w0 = w_sb[:, hc * DPG : hc * DPG + P]
                    w1 = w_sb[:, hc * DPG + P : (hc + 1) * DPG]
                    nc.tensor.matmul(pp[0][:], w0, xt[:, :QB], **st)
                    nc.tensor.matmul(pp[1][:], w0, xt[:, QB:], **st)
                    nc.tensor.matmul(pp[2][:], w1, xt[:, :QB], **st)
                    nc.tensor.matmul(pp[3][:], w1, xt[:, QB:], **st)
                for j in range(4):
                    dst = dst0 if j < 2 else dst1
                    bc = bcol + (0 if j < 2 else 1)
                    qb = pr * 2 + (j % 2)
                    nc.vector.tensor_scalar(
                        dst[:, qb * QB : (qb + 1) * QB], pp[j][:],
                        bqk_sb[:, bc : bc + 1], None, ADD,
                    )

            def v_transposes(pr, psA, part=None):
                scs = list(range(pr * NKC // 2, (pr + 1) * NKC // 2))
                if part is not None:
                    scs = scs[part * 2 : (part + 1) * 2]
                for sc in scs:
                    for half, src_t in ((0, vT0), (1, vT1)):
                        vtr = psA.tile([P, P], BF16, tag="vtr", bufs=2)
                        nc.tensor.transpose(
                            vtr[:], src_t[:, sc * P : (sc + 1) * P],
                            id_bf[:],
                        )
                        for j in range(2):
                            h = half * 2 + j
                            off = (sc * HPG + h) * VA_PAD
                            nc.vector.tensor_copy(
                                va_sb[:, off : off + HD],
                                vtr[:, j * HD : (j + 1) * HD],
                            )

            def scores_round(r, pool, tag, split=False):
                h, pr, kc = rounds[r]
                qT_t = qT0 if h < 2 else qT1
                kT_t = kT0 if h < 2 else kT1
                rows = slice((h % 2) * HD, (h % 2) * HD + HD)
                p2 = ppool.tile([P, 2 * QB], BF16, tag="p", name="p2")
                if split:
                    # single-bank tiles (PSUM is tight during projections)
                    for i in range(2):
                        qb = pr * 2 + i
                        s1 = pool.tile([P, QB], F32, tag=tag, name="s1")
                        nc.tensor.matmul(
                            s1[:],
                            kT_t[rows, kc * P : (kc + 1) * P],
                            qT_t[rows, qb * QB : (qb + 1) * QB],
                            start=True,
                            stop=True,
                        )
                        nc.scalar.activation(
                            p2[:, i * QB : (i + 1) * QB], s1[:], Exp,
                            bias=mk_sb[:, kc : kc + 1], scale=0.125,
                        )
                    pq[r] = p2
                    return
                s2 = pool.tile([P, 2 * QB], F32, tag=tag, name="s2")
                for i in range(2):
                    qb = pr * 2 + i
                    nc.tensor.matmul(
                        s2[:, i * QB : (i + 1) * QB],
                        kT_t[rows, kc * P : (kc + 1) * P],
                        qT_t[rows, qb * QB : (qb + 1) * QB],
                        start=True,
                        stop=True,
                    )
                nc.scalar.activation(
                    p2[:], s2[:], Exp,
                    bias=mk_sb[:, kc : kc + 1], scale=0.125,
                )
                pq[r] = p2

            # ---- phase 1: projections; the first NINJ attention
            # scores+exp rounds are interleaved into the v phase so the
            # scalar engine starts early ----
            with tc.tile_pool(name="psA", bufs=2, space="PSUM") as psA:
                for hc in range(NHC):
                    sl = slice(hc * DPG, (hc + 1) * DPG)
                    nc.sync.dma_start(wk_sb[:, sl], wk_d[hc * P : (hc + 1) * P, :])
                proj_pair(xk_d, wk_sb, 2, kT0, kT1, 0, psA)
                for hc in range(NHC):
                    sl = slice(hc * DPG, (hc + 1) * DPG)
                    nc.sync.dma_start(wq_sb[:, sl], wq_d[hc * P : (hc + 1) * P, :])
                proj_pair(xk_d, wk_sb, 2, kT0, kT1, 1, psA)
                proj_pair(xq_d, wq_sb, 0, qT0, qT1, 0, psA)
                for hc in range(NHC):
                    sl = slice(hc * DPG, (hc + 1) * DPG)
                    nc.sync.dma_start(wv_sb[:, sl], wv_d[hc * P : (hc + 1) * P, :])
                with tc.tile_pool(name="psS1", bufs=2, space="PSUM") as psS1:
                    inj = iter(range(NINJ))

                    def inject(hc):
                        if hc % 2 == 0:
                            r = next(inj, None)
                            if r is not None:
                                scores_round(r, psS1, "s2a", split=True)

                    proj_pair(xq_d, wq_sb, 0, qT0, qT1, 1, psA,
                              inject=inject)
                    proj_pair(xv_d, wv_sb, 4, vT0, vT1, 0, psA,
                              inject=inject)
                    for part in range(4):
                        inject(0)
                        v_transposes(0, psA, part)
                    proj_pair(xv_d, wv_sb, 4, vT0, vT1, 1, psA,
                              inject=inject)
                    for part in range(4):
                        inject(0)
                        v_transposes(1, psA, part)
                    for r in inj:
                        scores_round(r, psS1, "s2a", split=True)

            # ---- phase 2: flat software-pipelined attention ----
            with (
                tc.tile_pool(name="psS", bufs=3, space="PSUM") as psS,
                tc.tile_pool(name="psC", bufs=2, space="PSUM") as psC,
            ):
                ctxs = {}

                def ctx_round(r2):
                    h2, pr2, kc2 = rounds[r2]
                    if (h2, pr2) not in ctxs:
                        ctxs[(h2, pr2)] = [
                            psC.tile([VA_PAD, QB], F32, tag="ctx",
                                     name=f"ctx{h2}_{pr2}_{i}")
                            for i in range(2)
                        ]
                    off = (kc2 * HPG + h2) * VA_PAD
                    for i in range(2):
                        nc.tensor.matmul(
                            ctxs[(h2, pr2)][i][:],
                            va_sb[:, off : off + VA_PAD],
                            pq[r2][:, i * QB : (i + 1) * QB],
                            start=(kc2 == 0),
                            stop=(kc2 == NKC - 1),
                        )
                    pq[r2] = None
                    if kc2 == NKC - 1:
                        epilogue(ctxs.pop((h2, pr2)), h2, pr2)

                def epilogue(ctx2, h, pair):
                    for i in range(2):
                        qb = pair * 2 + i
                        cs = spool.tile([VA_W, QB], F32, tag="cs")
                        nc.vector.tensor_copy(cs[:], ctx2[i][:VA_W, :])
                        for t in range(NQB):
                            tr = psC.tile([P, VA_W], F32, tag="ctx")
                            nc.tensor.transpose(
                                tr[:],
                                cs[:, t * P : (t + 1) * P],
                                id_sb[:VA_W, :VA_W],
                            )
                            rec = spool.tile([P, 1], F32, tag="rec")
                            nc.vector.reciprocal(rec[:], tr[:, HD : HD + 1])
                            o_t = spool.tile([P, HD], F32, tag="o")
                            nc.vector.tensor_scalar(
                                o_t[:], tr[:, :HD], rec[:], None, MULT,
                            )
                            nc.sync.dma_start(
                                out_d[
                                    qb * QB + t * P : qb * QB + (t + 1) * P,
                                    h * HD : (h + 1) * HD,
                                ],
                                o_t[:],
                            )

                done = 0  # ctx rounds emitted so far
                for r in range(NINJ, NR):
                    scores_round(r, psS, "s2")
                    # catch up the ctx stream toward lag 2
                    want = r - 1
                    n_emit = 2 if done + 2 <= want else (1 if done < want else 0)
                    for _ in range(n_emit):
                        ctx_round(done)
                        done += 1
                while done < NR:
                    ctx_round(done)
                    done += 1

    nc.compile()
    return nc


def _in_maps(query, key, value, attention_mask, Wq, bq, Wk, bk, Wv, bv):
    import ml_dtypes

    bf16 = ml_dtypes.bfloat16
    q = np.asarray(query, np.float32)
    k = np.asarray(key, np.float32)
    v = np.asarray(value, np.float32)
    m = np.asarray(attention_mask, np.float32)
    Wq = np.asarray(Wq, np.float32)
    Wk = np.asarray(Wk, np.float32)
    Wv = np.asarray(Wv, np.float32)
    bq = np.asarray(bq, np.float32)
    bk = np.asarray(bk, np.float32)
    bv = np.asarray(bv, np.float32)

    xT = [
        (
            np.ascontiguousarray(q[b].T).astype(bf16),
            np.ascontiguousarray(k[b].T).astype(bf16),
            np.ascontiguousarray(v[b].T).astype(bf16),
        )
        for b in range(B)
    ]
    maps = []
    for c in range(NCORES):
        b, hg = divmod(c, GROUPS)
        hs = hg * DPG
        he = hs + DPG
        bqs, bks, bvs = bq[hs:he], bk[hs:he], bv[hs:he]
        bqk = np.stack(
            [bqs[:P], bqs[P:], bks[:P], bks[P:], bvs[:P], bvs[P:]], axis=1
        ).astype(np.float32)
        maps.append(
            {
                "xq": xT[b][0],
                "xk": xT[b][1],
                "xv": xT[b][2],
                "wq": np.ascontiguousarray(Wq[hs:he, :].T).astype(bf16),
                "wk": np.ascontiguousarray(Wk[hs:he, :].T).astype(bf16),
                "wv": np.ascontiguousarray(Wv[hs:he, :].T).astype(bf16),
                "bqk": np.ascontiguousarray(bqk),
                "mk": np.ascontiguousarray(m[b, 0, 0].reshape(NKC, P).T),
                "ident": np.eye(P, dtype=np.float32),
            }
        )
    return maps


def kernel(query, key, value, attention_mask, Wq, bq, Wk, bk, Wv, bv):
    from concourse.bass_utils import run_bass_kernel_spmd

    nc = _build()
    maps = _in_maps(
        query, key, value, attention_mask, Wq, bq, Wk, bk, Wv, bv
    )
    res = run_bass_kernel_spmd(nc, maps, core_ids=list(range(NCORES)))
    out = np.empty((B, S, H), np.float32)
    for c in range(NCORES):
        b, hg = divmod(c, GROUPS)
        out[b, :, hg * DPG : (hg + 1) * DPG] = res.results[c]["out"]
    return out
